# revision 6
# baseline (speedup 1.0000x reference)
"""Trainium2 Bass kernel for nn_MultiHeadAttention_79018808312395.

Multi-head attention (sigmoid-then-softmax variant) over 8 NeuronCores:

    q = queries @ Wq.T + bq ; k, v likewise
    scores = q k^T / sqrt(D) per (batch, head)
    w = sigmoid(scores)            (1 - sigmoid if indicator != 0)
    attn = softmax(w)
    out = (attn @ v) @ Wo.T + bo

Shapes: B=2, S=2048, E=1024, H=16, D=64.

Sharding (data-parallel over batch x query-slice; no collectives needed):
  core c owns batch b = c // 4 and query rows [qs, qs+512), qs = (c%4)*512.
  Each core projects its batch's FULL keys/values (k/v work is duplicated
  4x across the cores of a batch), projects its own query slice, computes
  attention for all 16 heads on its 512 queries, and the output projection
  for its disjoint [512, 1024] slice of the result.  Unshard = pure concat.

Kernel internals (per core):
  - All projections consume X.T tiles; X.T and W.T are produced on-chip via
    PE (tensor-engine) transposes of naturally-loaded tiles.
  - Scores are computed transposed, [k_tok(part), q(free)], so the softmax
    denominator comes for free as an extra ones-column in the attn@v matmul
    (row 64 of the [65, 512] psum accumulates sum_k f[k, q]).
  - sigmoid+exp is done as tanh then exp — both live in ACT's
    "exp_and_others" table set, so there are no table switches:
      softmax(sigmoid(s/8)) == softmax_weights exp(0.5*tanh(s/16))
    (shift invariance absorbs the +0.5 constant; sign of the scale handles
    the indicator branch since 1 - sigmoid(x) = sigmoid(-x)).
  - The softmax division is deferred past attn@v: o = (f @ v) * (1/sum),
    applied to the tiny [64, 512] o-tiles; bv is folded in after the divide
    (softmax rows sum to 1).
  - Projection matmuls run as float32r (full-rate fp32 mode of the PE);
    attention operands are stored bf16 (psum accumulation stays fp32).

This file is self-contained: it includes the workarounds for this
container's walrus build (max one semaphore wait per instruction).
"""

import json
import types

import numpy as np

import concourse.bass as bass
import concourse.mybir as mybir
import concourse.tile as tile
from concourse.vector_clock import ScopedClock

B, S, E, H = 2, 2048, 1024, 16
D = E // H          # 64
N_CORES = 8
QS = S * B // N_CORES   # 512 query rows per core
F32 = mybir.dt.float32
F32R = mybir.dt.float32r
BF16 = mybir.dt.bfloat16
AF = mybir.ActivationFunctionType

# knobs
USE_FP32R = True      # bitcast fp32 matmul operands to float32r (4x faster)
ATTN_BF16 = True      # store qT/kT/v/f in bf16 for the attention matmuls
TANH_INPLACE = True   # tanh written back into the scores psum tile


# ---------------------------------------------------------------------------
# walrus workarounds: this container's walrus accepts at most ONE semaphore
# wait per instruction; Tile emits several (epilogue drain + any instruction
# whose inputs come from two engines).  Fix (a) the epilogue by emitting
# per-proc single-wait NOPs, (b) everything else by splitting multi-wait
# instructions into preceding single-wait NoOps in the serialized BIR.
# ---------------------------------------------------------------------------

class PatchedTileContext(tile.TileContext):
    def _drain_and_barrier(self, tick_clock, wait_clock):
        vc = tick_clock.global_clock
        for proc in range(len(vc)):
            t = vc[proc]
            if t <= 0:
                continue
            nop = self.nc.sync.nop()
            sc = ScopedClock()
            sc.require_at_least(None, proc, t)
            wait_clock.add_sem_waits(nop.ins, sc)
        self.nc.sync.drain()
        self.nc.all_engine_barrier()
        assert self.sems is not None
        popped = self.nc._tile_sem_poison_stack.pop()
        assert popped is self._sem_poison
        self.nc.clear_and_free_semaphores(list(self.sems.allocated().values()))
        self.nc.all_engine_barrier()


def _split_multiwait_bir(d: dict) -> dict:
    ctr = 0
    for fn in d.get("functions", []):
        for bb in fn.get("blocks", []):
            out = []
            for inst in bb.get("instructions", []):
                si = inst.get("sync_info")
                if si:
                    ow = si.get("on_wait") or []
                    if len(ow) > 1:
                        for w in ow[:-1]:
                            ctr += 1
                            out.append({
                                "debug": inst.get("debug", 0),
                                "engine": inst["engine"],
                                "ins": [],
                                "name": f"IWS-{ctr}",
                                "opcode": "NoOp",
                                "outs": [],
                                "sync_info": {"on_update": [], "on_wait": [w]},
                            })
                        si["on_wait"] = [ow[-1]]
                    ou = si.get("on_update") or []
                    if len(ou) > 1:
                        raise RuntimeError(
                            f"{inst.get('name')}: {len(ou)} sem updates "
                            "(walrus caps at 1)"
                        )
                out.append(inst)
            bb["instructions"] = out
    return d


def _install_bir_wait_splitter(nc):
    orig = nc.to_json_bytes

    def to_json_bytes(self):
        return json.dumps(_split_multiwait_bir(json.loads(orig()))).encode()

    nc.to_json_bytes = types.MethodType(to_json_bytes, nc)
    return nc


# ---------------------------------------------------------------------------
# kernel builder (SPMD program, one NeuronCore's view)
# ---------------------------------------------------------------------------

PDT = F32R if USE_FP32R else F32   # dtype of tiles feeding fp32 matmuls


def _mm(nc, out, lhsT, rhs, **kw):
    return nc.tensor.matmul(out, lhsT, rhs, **kw)


def build_kernel(reps: int = 1):
    adt = BF16 if ATTN_BF16 else F32
    nc = bass.Bass()

    xq = nc.declare_dram_parameter("xq", [QS, E], F32, isOutput=False)
    xk = nc.declare_dram_parameter("xk", [S, E], F32, isOutput=False)
    xv = nc.declare_dram_parameter("xv", [S, E], F32, isOutput=False)
    wts = {
        n: nc.declare_dram_parameter(n, [E, E], F32, isOutput=False)
        for n in ("wq", "wk", "wv", "wo")
    }
    bq_r = nc.declare_dram_parameter("bq_r", [128, 8], F32, isOutput=False)
    bk_r = nc.declare_dram_parameter("bk_r", [128, 8], F32, isOutput=False)
    bv_r = nc.declare_dram_parameter("bv_r", [128, 8], F32, isOutput=False)
    bo_row = nc.declare_dram_parameter("bo_row", [1, E], PDT, isOutput=False)
    sc_sign = nc.declare_dram_parameter("sc_sign", [128, 1], F32, isOutput=False)
    ident = nc.declare_dram_parameter("ident", [128, 128], F32, isOutput=False)
    ones_r = nc.declare_dram_parameter("ones_r", [1, 128], PDT, isOutput=False)
    y = nc.declare_dram_parameter("y", [QS, E], F32, isOutput=True)

    NK = S // 128        # 16 k-token chunks
    KT_T = S // 512      # 4 token tiles for k/v projection

    with PatchedTileContext(nc) as tc:
      from contextlib import ExitStack
      for _rep in range(reps):
        with ExitStack() as ctx:
            const = ctx.enter_context(tc.tile_pool(name=f"const{_rep}", bufs=1))
            natp = ctx.enter_context(tc.tile_pool(name=f"natp{_rep}", bufs=2))
            xtp = ctx.enter_context(tc.tile_pool(name=f"xtp{_rep}", bufs=1))
            xvp = ctx.enter_context(tc.tile_pool(name=f"xvp{_rep}", bufs=2))
            big = ctx.enter_context(tc.tile_pool(name=f"big{_rep}", bufs=1))
            tp = ctx.enter_context(tc.tile_pool(name=f"tp{_rep}", bufs=2))
            fp_ = ctx.enter_context(tc.tile_pool(name=f"fp{_rep}", bufs=2))
            rcp = ctx.enter_context(tc.tile_pool(name=f"rcp{_rep}", bufs=2))
            yp = ctx.enter_context(tc.tile_pool(name=f"yp{_rep}", bufs=2))
            # psum pools: 1+1+4+1+1 = 8 banks exactly
            ptp = ctx.enter_context(tc.tile_pool(name=f"ptp{_rep}", bufs=1, space="PSUM"))
            ppp = ctx.enter_context(tc.tile_pool(name=f"ppp{_rep}", bufs=1, space="PSUM"))
            psp = ctx.enter_context(tc.tile_pool(name=f"psp{_rep}", bufs=1, space="PSUM"))
            pop = ctx.enter_context(tc.tile_pool(name=f"pop{_rep}", bufs=1, space="PSUM"))
            pbp = ctx.enter_context(tc.tile_pool(name=f"pbp{_rep}", bufs=1, space="PSUM"))

            ident_sb = const.tile([128, 128], F32, tag="ident")
            nc.sync.dma_start(ident_sb[:], ident[:])
            ones_sb = const.tile([1, 128], PDT, tag="ones")
            nc.sync.dma_start(ones_sb[:], ones_r[:])
            scs_sb = const.tile([128, 1], F32, tag="scs")
            nc.sync.dma_start(scs_sb[:], sc_sign[:])
            bq_sb = const.tile([128, 8], F32, tag="bq")
            nc.sync.dma_start(bq_sb[:], bq_r[:])
            bk_sb = const.tile([128, 8], F32, tag="bk")
            nc.sync.dma_start(bk_sb[:], bk_r[:])
            bv_sb = const.tile([128, 8], F32, tag="bv")
            nc.sync.dma_start(bv_sb[:], bv_r[:])
            bo_sb = const.tile([1, E], PDT, tag="bo")
            nc.sync.dma_start(bo_sb[:], bo_row[:])

            # attention operands, resident
            kT = big.tile([128, 8, S], adt, tag="kT")        # [feat, fo, tok]
            vA = big.tile([128, NK, H, 65], adt, tag="vA")   # v + ones col
            qT = big.tile([128, 8, QS], adt, tag="qT")
            oall = big.tile([128, 8, QS], PDT, tag="oall")   # normalized attn out
            nc.vector.memset(vA[:, :, :, 64:65], 1.0)

            def transpose_w(wdram, pool, tag):
                """load W [E,E] natural, PE-transpose into [128, ci, of]."""
                wT = pool.tile([128, 8, E], PDT, tag=tag)
                for co in range(8):
                    nat = natp.tile([128, E], F32, tag="wnat")
                    nc.sync.dma_start(nat[:], wdram[co * 128:(co + 1) * 128, :])
                    for g in range(2):
                        pt = ptp.tile([128, 4, 128], F32, tag="pt")
                        for c4 in range(4):
                            ci = g * 4 + c4
                            nc.tensor.transpose(
                                pt[:, c4, :],
                                nat[:, ci * 128:(ci + 1) * 128],
                                ident_sb[:],
                            )
                        nc.vector.tensor_copy(
                            wT[:, g * 4:(g + 1) * 4, co * 128:(co + 1) * 128],
                            pt[:],
                        )
                return wT

            def transpose_x_tile(xdram, row0, dst, dst_tok0):
                """one [128, E] natural row-block -> dst[:, ci, dst_tok0+128)."""
                nat = natp.tile([128, E], F32, tag="xnat")
                nc.sync.dma_start(nat[:], xdram[row0:row0 + 128, :])
                for g in range(2):
                    pt = ptp.tile([128, 4, 128], F32, tag="pt")
                    for c4 in range(4):
                        ci = g * 4 + c4
                        nc.tensor.transpose(
                            pt[:, c4, :],
                            nat[:, ci * 128:(ci + 1) * 128],
                            ident_sb[:],
                        )
                    nc.vector.tensor_copy(
                        dst[:, g * 4:(g + 1) * 4, dst_tok0:dst_tok0 + 128],
                        pt[:],
                    )

            # ---- phase 1a: qT projection ----------------------------------
            with tc.tile_pool(name=f"wp_q{_rep}", bufs=1) as wpq:
                wqT = transpose_w(wts["wq"], wpq, "wqT")
                xT = xtp.tile([128, 8, 512], PDT, tag="xT")
                for tb in range(4):
                    transpose_x_tile(xq, tb * 128, xT, tb * 128)
                for fo in range(8):
                    pp = ppp.tile([128, 512], F32, tag="pp")
                    for ci in range(8):
                        _mm(nc, pp[:], wqT[:, ci, fo * 128:(fo + 1) * 128],
                            xT[:, ci, :], start=(ci == 0), stop=(ci == 7))
                    nc.scalar.activation(qT[:, fo, :], pp[:], AF.Identity,
                                         bias=bq_sb[:, fo:fo + 1])

            # ---- phase 1b: v projection (natural layout, into vA) ---------
            with tc.tile_pool(name=f"wp_v{_rep}", bufs=1) as wpv:
                wvT = transpose_w(wts["wv"], wpv, "wvT")
                for tcn in range(NK):
                    xvT = xvp.tile([128, 8, 128], PDT, tag="xvT")
                    nat = natp.tile([128, E], F32, tag="xnat")
                    nc.sync.dma_start(nat[:], xv[tcn * 128:(tcn + 1) * 128, :])
                    for g in range(2):
                        pt = ptp.tile([128, 4, 128], F32, tag="pt")
                        for c4 in range(4):
                            ci = g * 4 + c4
                            nc.tensor.transpose(
                                pt[:, c4, :],
                                nat[:, ci * 128:(ci + 1) * 128],
                                ident_sb[:],
                            )
                        nc.vector.tensor_copy(xvT[:, g * 4:(g + 1) * 4, :], pt[:])
                    for j in range(2):
                        pv = ppp.tile([128, 512], F32, tag="pp")
                        for ci in range(8):
                            _mm(nc, pv[:], xvT[:, ci, :],
                                wvT[:, ci, j * 512:(j + 1) * 512],
                                start=(ci == 0), stop=(ci == 7))
                        # scatter 8 heads x 64 feats into vA (cast to adt)
                        nc.vector.tensor_copy(
                            vA[:, tcn, j * 8:(j + 1) * 8, 0:64],
                            pv[:].rearrange("p (h d) -> p h d", d=64),
                        )

            # ---- phase 1c: kT projection ----------------------------------
            with tc.tile_pool(name=f"wp_k{_rep}", bufs=1) as wpk:
                wkT = transpose_w(wts["wk"], wpk, "wkT")
                for t in range(KT_T):
                    xT = xtp.tile([128, 8, 512], PDT, tag="xT")
                    for tb in range(4):
                        transpose_x_tile(xk, t * 512 + tb * 128, xT, tb * 128)
                    for fo in range(8):
                        pp = ppp.tile([128, 512], F32, tag="pp")
                        for ci in range(8):
                            _mm(nc, pp[:], wkT[:, ci, fo * 128:(fo + 1) * 128],
                                xT[:, ci, :], start=(ci == 0), stop=(ci == 7))
                        nc.scalar.activation(kT[:, fo, t * 512:(t + 1) * 512],
                                             pp[:], AF.Identity,
                                             bias=bk_sb[:, fo:fo + 1])

            # ---- phase 1d: woT (needed at the end) ------------------------
            wpo = ctx.enter_context(tc.tile_pool(name=f"wp_o{_rep}", bufs=1))
            woT = transpose_w(wts["wo"], wpo, "woT")

            # ---- phase 2: attention ---------------------------------------
            for h in range(H):
                ci_h, off = h // 2, 64 * (h % 2)
                po = pop.tile([65, 512], F32, tag="po")
                for g in range(4):
                    ps = psp.tile([128, 4, 512], F32, tag="ps")
                    for k4 in range(4):
                        kc = g * 4 + k4
                        _mm(nc, ps[:, k4, :],
                            kT[off:off + 64, ci_h, kc * 128:(kc + 1) * 128],
                            qT[off:off + 64, ci_h, :])
                    # tanh(+-scores/16), then f = exp(0.5*tanh)
                    if TANH_INPLACE:
                        nc.scalar.activation(ps[:], ps[:], AF.Tanh,
                                             scale=scs_sb[:, 0:1])
                        tin = ps
                    else:
                        tsb = tp.tile([128, 4, 512], adt, tag="tsb")
                        nc.scalar.activation(tsb[:], ps[:], AF.Tanh,
                                             scale=scs_sb[:, 0:1])
                        tin = tsb
                    fsb = fp_.tile([128, 4, 512], adt, tag="fsb")
                    nc.scalar.activation(fsb[:], tin[:], AF.Exp, scale=0.5)
                    for k4 in range(4):
                        kc = g * 4 + k4
                        _mm(nc, po[:], vA[:, kc, h, :], fsb[:, k4, :],
                            start=(kc == 0), stop=(kc == NK - 1))
                # normalize + bv
                rc = rcp.tile([1, 512], PDT, tag="rc")
                with nc.allow_low_precision(reason="1/sum rounded to fp32r"):
                    nc.vector.reciprocal(rc[:], po[64:65, :])
                pb = pbp.tile([64, 512], F32, tag="pb")
                _mm(nc, pb[:], ones_sb[0:1, 0:64], rc[:])
                pb_sb = rcp.tile([64, 512], F32, tag="pbs")
                nc.scalar.copy(pb_sb[:], pb[:])
                nc.vector.tensor_mul(oall[off:off + 64, ci_h, :],
                                     po[0:64, :], pb_sb[:])
                nc.vector.tensor_scalar_add(oall[off:off + 64, ci_h, :],
                                            oall[off:off + 64, ci_h, :],
                                            bv_sb[off:off + 64, ci_h:ci_h + 1])

            # ---- phase 3: output projection -------------------------------
            for tcn in range(QS // 128):
                for j in range(2):
                    py = ppp.tile([128, 512], F32, tag="pp")
                    for ci in range(8):
                        _mm(nc, py[:], oall[:, ci, tcn * 128:(tcn + 1) * 128],
                            woT[:, ci, j * 512:(j + 1) * 512],
                            start=(ci == 0), stop=False)
                    _mm(nc, py[:], ones_sb[:], bo_sb[0:1, j * 512:(j + 1) * 512],
                        start=False, stop=True)
                    ysb = yp.tile([128, 512], F32, tag="ysb")
                    nc.scalar.copy(ysb[:], py[:])
                    nc.sync.dma_start(
                        y[tcn * 128:(tcn + 1) * 128, j * 512:(j + 1) * 512],
                        ysb[:])

    _install_bir_wait_splitter(nc)
    return nc


# ---------------------------------------------------------------------------
# host-side shard / run / unshard
# ---------------------------------------------------------------------------

_cached = {}


def _get_nc(reps: int = 1):
    key = ("nc", reps)
    if key not in _cached:
        _cached[key] = build_kernel(reps)
    return _cached[key]


def make_in_maps(queries, keys, values, Wq, bq, Wk, bk, Wv, bv, Wo, bo,
                 indicator):
    queries = np.ascontiguousarray(np.asarray(queries, dtype=np.float32))
    keys = np.ascontiguousarray(np.asarray(keys, dtype=np.float32))
    values = np.ascontiguousarray(np.asarray(values, dtype=np.float32))
    sign = np.float32(-0.0625) if int(indicator) != 0 else np.float32(0.0625)
    shared = {
        "wq": np.ascontiguousarray(np.asarray(Wq, np.float32)),
        "wk": np.ascontiguousarray(np.asarray(Wk, np.float32)),
        "wv": np.ascontiguousarray(np.asarray(Wv, np.float32)),
        "wo": np.ascontiguousarray(np.asarray(Wo, np.float32)),
        "bq_r": np.ascontiguousarray(np.asarray(bq, np.float32).reshape(8, 128).T),
        "bk_r": np.ascontiguousarray(np.asarray(bk, np.float32).reshape(8, 128).T),
        "bv_r": np.ascontiguousarray(np.asarray(bv, np.float32).reshape(8, 128).T),
        "bo_row": np.ascontiguousarray(np.asarray(bo, np.float32).reshape(1, E)),
        "sc_sign": np.full((128, 1), sign, np.float32),
        "ident": np.eye(128, dtype=np.float32),
        "ones_r": np.ones((1, 128), np.float32),
    }
    in_maps = []
    for c in range(N_CORES):
        b, qs = c // 4, (c % 4) * QS
        m = dict(shared)
        m["xq"] = np.ascontiguousarray(queries[b, qs:qs + QS, :])
        m["xk"] = keys[b]
        m["xv"] = values[b]
        in_maps.append(m)
    return in_maps


def unshard(results):
    out = np.empty((B, S, E), np.float32)
    for c in range(N_CORES):
        b, qs = c // 4, (c % 4) * QS
        out[b, qs:qs + QS, :] = results[c]["y"]
    return out


def kernel(**inputs) -> np.ndarray:
    from concourse.bass_utils import run_bass_kernel_spmd
    nc = _get_nc()
    in_maps = make_in_maps(**inputs)
    res = run_bass_kernel_spmd(nc, in_maps, list(range(N_CORES)))
    return unshard(res.results)
